# revision 32
# baseline (speedup 1.0000x reference)
"""GQA attention (B=2, S=2048, H=2048, 32 heads / 8 KV groups, rope, causal-masked
softmax, output projection) distributed over 8 Trainium2 NeuronCores.

Sharding: data parallel over batch (2) x tensor parallel over KV groups (4 group-pairs).
Core c handles batch c//4 and KV groups {2*(c%4), 2*(c%4)+1} (= 8 q heads). Each core
computes its partial output projection (attn_out_shard @ wo_cols_shard.T); the host
sums the 4 partials per batch (the "all-reduce") and adds bo.

On-core layout trick: head dims of Q/K are permuted per head to [evens | odds] so rope
becomes block elementwise ops; scores are computed transposed (s_k on partitions) so
softmax denominators come free as an extra ones-column in V and attn@V feeds the
output projection without transposes.

Perf notes vs the fp32r baseline (664-800us):
- All matmul operands are float16: 1 cycle/row streaming + FWL weight loads
  (fp32/f32r measured ~630ns per N=512 matmul on HW; fp16 ~220ns).
- PSUM evictions for rope go through the Scalar engine (ACT copy) freeing DVE;
  rope/mask elementwise ops run in fp16 (2x DVE packing).
- exp() is restricted to the causally-valid column range of diagonal tiles
  (gpsimd memsets the masked prefix), and the masked multiply uses a single
  128x128 triangular tile.
- Softmax reciprocal uses reciprocal_approx_fast (~5x faster than DVE
  reciprocal; 18 good bits is plenty here).
- Projections (phase P, chunk sc) are interleaved with attention (phase A,
  chunk qt=sc): A(qt) only needs K/V tiles kt<4(qt+1) = P(0..qt), so exp work
  spreads across the kernel and PSUM pools alternate within 8 banks.
"""
import sys

for _p in ("/opt/trn_rl_repo",):
    if _p not in sys.path:
        sys.path.append(_p)

import numpy as np

S = 2048
H = 2048
HD = 64
NQT = 4          # s_q tiles of 512
NKT = 16         # s_k tiles of 128

_CACHE = {}


def _build(mode, has_bq, has_bk, has_bv):
    import concourse.bass as bass  # noqa: F401
    import concourse.mybir as mybir
    import concourse.tile as tile
    from concourse import bacc
    from concourse.masks import make_identity

    f32 = mybir.dt.float32
    f16 = mybir.dt.bfloat16
    AF = mybir.ActivationFunctionType
    ALU = mybir.AluOpType

    nc = bacc.Bacc("TRN2", target_bir_lowering=False, debug=False)
    xT = nc.dram_tensor("xT", [H, S], f16, kind="ExternalInput")
    wqT = nc.dram_tensor("wqT", [H, 512], f16, kind="ExternalInput")
    wkvT = nc.dram_tensor("wkvT", [H, 256], f16, kind="ExternalInput")
    woR = nc.dram_tensor("woR", [512, H], f16, kind="ExternalInput")
    COSd = nc.dram_tensor("COSx", [128, S], f16, kind="ExternalInput")
    SINd = nc.dram_tensor("SINx", [128, S], f16, kind="ExternalInput")
    PSWd = nc.dram_tensor("PSW", [128, 128], f16, kind="ExternalInput")
    outd = nc.dram_tensor("out", [S, H], f16, kind="ExternalOutput")
    trid = nc.dram_tensor("tri", [128, 128], f16, kind="ExternalInput") if mode == "causal" else None
    maskd = nc.dram_tensor("maskT", [S, S], f32, kind="ExternalInput") if mode == "generic" else None
    bqd = nc.dram_tensor("bq", [512, 1], f32, kind="ExternalInput") if has_bq else None
    bkvd = nc.dram_tensor("bkv", [256, 1], f32, kind="ExternalInput") if (has_bk or has_bv) else None

    with tile.TileContext(nc) as tc:
        with (
            tc.tile_pool(name="const", bufs=1) as cstp,
            tc.tile_pool(name="wts", bufs=1) as wts,
            tc.tile_pool(name="xs", bufs=18) as xsp,
            tc.tile_pool(name="per", bufs=1) as per,
            tc.tile_pool(name="rtmp", bufs=2) as rtp,
            tc.tile_pool(name="et", bufs=3) as etp,
            tc.tile_pool(name="outs", bufs=2) as outp,
            tc.tile_pool(name="mks", bufs=2) as mkp,
        ):
            # one-time bulk constants go on the scalar-engine HWDGE queue so
            # they don't delay the first projection x/weight loads (sync queue)
            COS = cstp.tile([128, S], f16, tag="cos")
            SIN = cstp.tile([128, S], f16, tag="sin")
            nc.scalar.dma_start(COS[:], COSd[:])
            nc.scalar.dma_start(SIN[:], SINd[:])
            PSW = cstp.tile([128, 128], f16, tag="psw")
            nc.scalar.dma_start(PSW[:], PSWd[:])
            ident = cstp.tile([128, 128], f16, tag="ident")
            make_identity(nc, ident[:])
            if mode == "causal":
                TRI = cstp.tile([128, 128], f16, tag="tri")
                nc.scalar.dma_start(TRI[:], trid[:])
            # bias tiles: partition dim max 128 -> split loads
            if has_bq:
                bq_t = [cstp.tile([128, 1], f32, tag=f"bq{m}", name=f"bq_t{m}") for m in range(4)]
                for m in range(4):
                    nc.scalar.dma_start(bq_t[m][:], bqd[128 * m:128 * (m + 1), :])
            if has_bk or has_bv:
                bk_t = cstp.tile([128, 1], f32, tag="bkt")
                bv_t = cstp.tile([128, 1], f32, tag="bvt")
                nc.sync.dma_start(bk_t[:], bkvd[0:128, :])
                nc.sync.dma_start(bv_t[:], bkvd[128:256, :])

            # resident weights: wq/wkv loads are interleaved with the first
            # phase-P chunk's x loads below (just-in-time); wo is needed only
            # by the first output projection so it rides the scalar queue
            wq_t = [wts.tile([128, 512], f16, tag=f"wq{k}", name=f"wq_t{k}") for k in range(16)]
            wkv_t = [wts.tile([128, 256], f16, tag=f"wkv{k}", name=f"wkv_t{k}") for k in range(16)]
            wo_t = [wts.tile([128, S], f16, tag=f"wor{k}", name=f"wo_t{k}") for k in range(4)]
            for k in range(4):
                nc.scalar.dma_start(wo_t[k][:], woR[128 * k:128 * (k + 1), :])

            # persistent intermediates
            QTrot = [per.tile([128, S], f16, tag=f"qtrot{m}", name=f"QTrot{m}") for m in range(4)]
            KTrot = per.tile([128, S], f16, tag="ktrot")
            # V with a ones column per kt-block: [g0 v64 | 1 | g1 v64 | 1] x 16 kt
            Vp = per.tile([128, 130 * NKT], f16, tag="vp")
            nc.gpsimd.memset(Vp[:], 1.0)  # ones columns at 130*kt+{64,129} survive
            # denominator staging: head (m,hloc) -> partition 32*m, cols 512*hloc
            den = per.tile([128, 1024], f32, tag="den")
            rden = per.tile([128, 1024], f32, tag="rden")
            rden16 = per.tile([128, 1024], f16, tag="rden16")
            nc.gpsimd.memset(den[:], 1.0)
            nc.gpsimd.memset(rden16[:], 1.0)
            ones1 = per.tile([1, 512], f32, tag="ones1")
            nc.gpsimd.memset(ones1[:], 1.0)
            # selector tiles: sel[m] has ones in partition-row 32m; lhsT for the
            # matmuls that broadcast den row 32m across 64 psum partitions
            sel = [per.tile([128, 64], f16, tag=f"sel{m}", name=f"sel{m}") for m in range(4)]
            for m in range(4):
                nc.gpsimd.memset(sel[m][:], 0.0)
                nc.gpsimd.memset(sel[m][32 * m:32 * m + 1, :], 1.0)
            VTt = per.tile([128, 512], f16, tag="vtt")

            for ph in range(4):
                # ---------- Phase P chunk: projections + rope + V transpose for sc=ph ----------
                sc = ph
                ssl = slice(512 * sc, 512 * (sc + 1))
                with tc.tile_pool(name=f"psP{ph}", bufs=1, space="PSUM") as psP:
                    qp = [psP.tile([128, 512], f32, tag=f"qp{m}", name=f"qp{m}") for m in range(4)]
                    kvK = psP.tile([128, 512], f32, tag="kvK")
                    kvV = psP.tile([128, 512], f32, tag="kvV")
                    for k in range(16):
                        if ph == 0:
                            # just-in-time weight loads ahead of their first use
                            nc.sync.dma_start(wq_t[k][:], wqT[128 * k:128 * (k + 1), :])
                            nc.sync.dma_start(wkv_t[k][:], wkvT[128 * k:128 * (k + 1), :])
                            xk = xsp.tile([128, 512], f16, tag="x")
                            nc.sync.dma_start(xk[:], xT[128 * k:128 * (k + 1), ssl])
                        else:
                            xk = xk_next[k]
                        st = (k == 0)
                        sp = (k == 15)
                        for m in range(4):
                            nc.tensor.matmul(qp[m][:], wq_t[k][:, 128 * m:128 * (m + 1)], xk[:], start=st, stop=sp)
                        nc.tensor.matmul(kvK[:], wkv_t[k][:, 0:128], xk[:], start=st, stop=sp)
                        nc.tensor.matmul(kvV[:], wkv_t[k][:, 128:256], xk[:], start=st, stop=sp)
                    # V: psum -> sbuf (+bias) fp16 (transposed + scattered after rope
                    # shift-matmuls so the PE has work while rope runs on DVE)
                    if has_bv:
                        nc.scalar.activation(VTt[:], kvV[:], AF.Identity, bias=bv_t[:])
                    else:
                        nc.scalar.copy(VTt[:], kvV[:])
                    # rope K -> KTrot, Q -> QTrot  (blocked layout [e|o] per head).
                    # The +-32 partition shift runs as a PSWAP matmul on the PE
                    # (reusing the projection's psum bank); DVE does 3 ops/tile.
                    rope_list = [(kvK, KTrot, "kvK", bk_t if has_bk else None)] + [
                        (qp[m], QTrot[m], f"qp{m}", bq_t[m] if has_bq else None) for m in range(4)]
                    for ri, (ps, dst, ptag, bias) in enumerate(rope_list):
                        qs = rtp.tile([128, 512], f16, tag="qs", name="qs")
                        if bias is not None:
                            nc.scalar.activation(qs[:], ps[:], AF.Identity, bias=bias[:])
                        else:
                            nc.scalar.copy(qs[:], ps[:])
                        rps = psP.tile([128, 512], f32, tag=ptag, name=f"rps_{ptag}")
                        nc.tensor.matmul(rps[:], PSW[:], qs[:], start=True, stop=True)
                        t1 = rtp.tile([128, 512], f16, tag="t1")
                        t2 = rtp.tile([128, 512], f16, tag="t2")
                        nc.vector.tensor_tensor(t1[:], qs[:], COS[:, ssl], ALU.mult)
                        nc.vector.tensor_tensor(t2[:], rps[:], SIN[:, ssl], ALU.mult)
                        nc.vector.tensor_tensor(dst[:, ssl], t1[:], t2[:], ALU.add)
                        if ri == 1:
                            # V transpose + scatter into Vp (PE work during rope)
                            for j in range(4):
                                kt = 4 * sc + j
                                vps = psP.tile([128, 128], f16, tag="vps", bufs=2)
                                nc.tensor.transpose(vps[:], VTt[:, 128 * j:128 * (j + 1)], ident[:])
                                # one copy: psum (128,(2,64)) -> Vp cols [130kt:+64] and [130kt+65:+129]
                                vdst = Vp[:, 130 * kt:130 * kt + 130].rearrange("p (two x) -> p two x", two=2)[:, :, 0:64]
                                src_ap = vps[:].rearrange("p (two x) -> p two x", two=2)
                                nc.vector.tensor_copy(vdst, src_ap)

                # ---------- Phase A chunk: attention + output projection for qt=ph ----------
                qt = ph
                qsl = slice(512 * qt, 512 * (qt + 1))
                n_kt = 4 * qt + 4 if mode == "causal" else NKT
                if ph < 3:
                    # prefetch the next projection chunk's x tiles while the
                    # sync DMA queue is idle (before this chunk's out stores)
                    xk_next = []
                    for k in range(16):
                        xk = xsp.tile([128, 512], f16, tag="x", name=f"xpre{k}")
                        nc.sync.dma_start(xk[:], xT[128 * k:128 * (k + 1), 512 * (ph + 1):512 * (ph + 2)])
                        xk_next.append(xk)
                with tc.tile_pool(name=f"psA{ph}", bufs=1, space="PSUM") as psA:
                    avq_all = []
                    pending_norm = []

                    def flush_norm(tag="op"):
                        # deferred normalization: emitted m-iterations late so
                        # the rcb broadcast matmuls never block the in-order
                        # tensor queue while the DVE recip chain runs. One
                        # full-tile recip+cast covers every staged head row
                        # (the custom DVE op misbehaves on partition slices).
                        if not pending_norm:
                            return
                        nc.vector.reciprocal_approx_fast(rden[:], den[:])
                        nc.vector.tensor_copy(rden16[:], rden[:])
                        while pending_norm:
                            mp = pending_norm.pop(0)
                            rcb = psA.tile([128, 512], f32, tag=tag, name=f"rcb{mp}", bufs=2)
                            nc.tensor.matmul(rcb[0:64, :], sel[mp][:], rden16[:, 0:512], start=True, stop=True)
                            nc.tensor.matmul(rcb[64:128, :], sel[mp][:], rden16[:, 512:1024], start=True, stop=True)
                            nc.vector.tensor_tensor(avq_all[mp][:], avq_all[mp][:], rcb[:], ALU.mult)

                    for m in range(4):
                        # two heads (m: group 0 rows 0:64, m+4: group 1 rows 64:128)
                        av = [psA.tile([128, 512], f32, tag="av", bufs=2, name=f"av{h}") for h in range(2)]
                        for pr in range(n_kt // 2):
                            sc2h = []
                            eTh = []
                            # both heads' score tiles allocated up-front; the
                            # matmuls alternate row groups 0-63/64-127 so the
                            # 16x(32x32) PE sub-arrays run both concurrently
                            for hloc in range(2):
                                sc2h.append(psA.tile([128, 1024], f32, tag="sc", bufs=2, name="sc2"))
                                eTh.append(etp.tile([128, 1024], f16, tag="eT", name="eT"))
                            for half in range(2):
                                kt = 2 * pr + half
                                # diagonal tiles only need the causally-valid
                                # q-column suffix
                                t = kt - 4 * qt
                                off = 128 * t if (mode == "causal" and 0 < t <= 3) else 0
                                for hloc in range(2):
                                    g = hloc
                                    qb = 64 * hloc
                                    nc.tensor.matmul(
                                        sc2h[hloc][:, 512 * half + off:512 * (half + 1)],
                                        KTrot[64 * g:64 * g + 64, 128 * kt:128 * (kt + 1)],
                                        QTrot[m][qb:qb + 64, slice(512 * qt + off, 512 * (qt + 1))],
                                        start=True, stop=True, tile_position=(64 * g, 0))
                            if pr == 1 and m == 2:
                                flush_norm()
                            for hloc in range(2):
                                sc2 = sc2h[hloc]
                                eT = eTh[hloc]
                                if mode == "generic":
                                    for half in range(2):
                                        kt = 2 * pr + half
                                        mk = mkp.tile([128, 512], f32, tag="mk", name="mk")
                                        nc.sync.dma_start(mk[:], maskd[128 * kt:128 * (kt + 1), qsl])
                                        stt = mkp.tile([128, 512], f32, tag="stt", name="stt")
                                        nc.vector.scalar_tensor_tensor(
                                            stt[:], sc2[:, 512 * half:512 * (half + 1)], 0.125, mk[:],
                                            ALU.mult, ALU.add)
                                        nc.scalar.activation(
                                            eT[:, 512 * half:512 * (half + 1)], stt[:], AF.Exp, scale=1.0)
                                elif mode == "causal" and 2 * pr >= 4 * qt:
                                    # diagonal pair: exp only the computed suffix of
                                    # each half (the masked prefix of eT is never
                                    # read: the AV matmul skips it); the 128-wide
                                    # diagonal block gets the triangular mask
                                    for half in range(2):
                                        kt = 2 * pr + half
                                        t = kt - 4 * qt
                                        off = 512 * half + 128 * t
                                        nc.scalar.activation(
                                            eT[:, off:512 * (half + 1)],
                                            sc2[:, off:512 * (half + 1)], AF.Exp, scale=0.125)
                                        nc.vector.tensor_tensor(
                                            eT[:, off:off + 128], eT[:, off:off + 128], TRI[:], ALU.mult)
                                else:
                                    nc.scalar.activation(eT[:], sc2[:], AF.Exp, scale=0.125)
                            for hloc in range(2):
                                g = hloc
                                eT = eTh[hloc]
                                for half in range(2):
                                    kt = 2 * pr + half
                                    t = kt - 4 * qt
                                    off = 128 * t if (mode == "causal" and 0 < t <= 3) else 0
                                    nc.tensor.matmul(
                                        av[hloc][0:65, off:512], Vp[:, 130 * kt + 65 * g:130 * kt + 65 * g + 65],
                                        eT[:, 512 * half + off:512 * (half + 1)],
                                        start=(kt == 0), stop=(kt == n_kt - 1))
                        avq = per.tile([128, 512], f16, tag=f"avtq{m}", bufs=2, name=f"avq{m}")
                        for hloc in range(2):
                            qb = 64 * hloc
                            # evict unnormalized AV and its denominator row; frees the psum bank
                            nc.vector.tensor_scalar_mul(avq[qb:qb + 64, :], av[hloc][0:64, :], 1.0)
                            nc.vector.tensor_tensor(den[32 * m:32 * m + 1, 512 * hloc:512 * (hloc + 1)],
                                                    av[hloc][64:65, :], ones1[:], ALU.mult)
                        avq_all.append(avq)
                        pending_norm.append(m)
                    flush_norm(tag="av")
                    # output projection for the 4 s-row-tiles of this qt block
                    for j in range(4):
                        mm = 4 * qt + j
                        for n in range(4):
                            nsl = slice(512 * n, 512 * (n + 1))
                            op = psA.tile([128, 512], f32, tag="op", bufs=2, name="op")
                            for k in range(4):
                                nc.tensor.matmul(op[:], avq_all[k][:, 128 * j:128 * (j + 1)],
                                                 wo_t[k][:, nsl], start=(k == 0), stop=(k == 3))
                            ot = outp.tile([128, 512], f16, tag="ot", name="ot")
                            nc.scalar.copy(ot[:], op[:])
                            nc.sync.dma_start(outd[128 * mm:128 * (mm + 1), nsl], ot[:])

    nc.compile()
    return nc


_PERM64 = np.concatenate([np.arange(0, 64, 2), np.arange(1, 64, 2)])
# Q-tile m holds local heads (m, m+4) so each head's partition base (0/64) matches
# its KV group's base in KTrot (group g at rows 64g) - matmul requires equal bases.
_HEADS_ORDER = np.array([0, 4, 1, 5, 2, 6, 3, 7])


def _prep_core(c, x, freqs_cis, mask, wq, bq, wk, bk, wv, bv, wo, mode,
               has_bq, has_bk, has_bv):
    import ml_dtypes
    b, gp = divmod(c, 4)
    f = ml_dtypes.bfloat16
    xT = np.ascontiguousarray(x[b].T, dtype=f)
    wq_c = wq[512 * gp:512 * (gp + 1)].reshape(8, 64, H)[_HEADS_ORDER][:, _PERM64, :].reshape(512, H)
    wqT = np.ascontiguousarray(wq_c.T, dtype=f)
    wk_c = wk[128 * gp:128 * (gp + 1)].reshape(2, 64, H)[:, _PERM64, :].reshape(128, H)
    wv_c = wv[128 * gp:128 * (gp + 1)]
    wkvT = np.ascontiguousarray(np.concatenate([wk_c, wv_c], 0).T, dtype=f)
    woR = wo[:, 512 * gp:512 * (gp + 1)].T.reshape(8, 64, H)[_HEADS_ORDER].reshape(512, H)
    woR = np.ascontiguousarray(woR, dtype=f)
    cosT = np.ascontiguousarray(freqs_cis[:, 0::2].T)   # (32, S)
    sinT = np.ascontiguousarray(freqs_cis[:, 1::2].T)
    COS = np.tile(cosT, (4, 1)).astype(f)
    # the rope partition shift (+sign) is a PSWAP matmul on the PE, so SIN is
    # all-positive: rot(x) = x*COS + (PSW.T @ x)*SIN with
    # PSW.T@x = [-odds | evens] per 64-row head block
    SIN = np.tile(sinT, (4, 1)).astype(f)
    PSW = np.zeros((128, 128), dtype=f)
    for base in (0, 64):
        for i in range(32):
            PSW[base + 32 + i, base + i] = -1.0   # even rows get -odds
            PSW[base + i, base + 32 + i] = 1.0    # odd rows get +evens
    m = {"xT": xT, "wqT": wqT, "wkvT": wkvT, "woR": woR,
         "COSx": np.ascontiguousarray(COS), "SINx": np.ascontiguousarray(SIN),
         "PSW": PSW}
    if mode == "causal":
        i = np.arange(128)[:, None]
        j = np.arange(128)[None, :]
        m["tri"] = (j >= i).astype(f)
    if mode == "generic":
        m["maskT"] = np.ascontiguousarray(mask.T, dtype=np.float32)
    if has_bq:
        bq_c = bq[512 * gp:512 * (gp + 1)].reshape(8, 64)[_HEADS_ORDER][:, _PERM64].reshape(512, 1)
        m["bq"] = np.ascontiguousarray(bq_c, dtype=np.float32)
    if has_bk or has_bv:
        bk_c = bk[128 * gp:128 * (gp + 1)].reshape(2, 64)[:, _PERM64].reshape(128)
        bv_c = bv[128 * gp:128 * (gp + 1)]
        m["bkv"] = np.ascontiguousarray(np.concatenate([bk_c, bv_c]).reshape(256, 1), dtype=np.float32)
    return m


def _detect_mode(mask):
    causal = np.where(np.tril(np.ones((S, S), dtype=bool)), np.float32(0.0), np.float32(-1e9))
    if np.array_equal(mask, causal):
        return "causal"
    if not np.any(mask):
        return "zeros"
    return "generic"


def _run(inputs, trace=False):
    from concourse import bass_utils
    x = np.asarray(inputs["x"], dtype=np.float32)
    freqs_cis = np.asarray(inputs["freqs_cis"], dtype=np.float32)
    mask = np.asarray(inputs["mask"], dtype=np.float32)
    wq = np.asarray(inputs["wq"], dtype=np.float32)
    bq = np.asarray(inputs["bq"], dtype=np.float32)
    wk = np.asarray(inputs["wk"], dtype=np.float32)
    bk = np.asarray(inputs["bk"], dtype=np.float32)
    wv = np.asarray(inputs["wv"], dtype=np.float32)
    bv = np.asarray(inputs["bv"], dtype=np.float32)
    wo = np.asarray(inputs["wo"], dtype=np.float32)
    bo = np.asarray(inputs["bo"], dtype=np.float32)

    mode = _detect_mode(mask)
    has_bq = bool(np.any(bq))
    has_bk = bool(np.any(bk))
    has_bv = bool(np.any(bv))
    key = (mode, has_bq, has_bk, has_bv)
    if key not in _CACHE:
        _CACHE[key] = _build(*key)
    nc = _CACHE[key]

    in_maps = [
        _prep_core(c, x, freqs_cis, mask, wq, bq, wk, bk, wv, bv, wo, mode,
                   has_bq, has_bk, has_bv)
        for c in range(8)
    ]
    res = bass_utils.run_bass_kernel_spmd(nc, in_maps, core_ids=list(range(8)), trace=trace)
    partials = np.stack([np.asarray(res.results[c]["out"], dtype=np.float32)
                         for c in range(8)], 0)  # (8, S, H)
    out = partials.reshape(2, 4, S, H).sum(axis=1) + bo[None, None, :]
    return out.astype(np.float32), res


def kernel(**inputs):
    out, _ = _run(inputs, trace=False)
    return out
